# revision 18
# baseline (speedup 1.0000x reference)
"""DiscreteLSTM Trainium2 kernel.

Reference math per step t:
    inp = h @ Wh + E[x_t] + b
    f,i,o = sigmoid(inp @ W{f,i,o} + b{f,i,o}); c = tanh(inp @ Wc + bc)
    h' = f*h + i*c ; y = o*tanh(h')

Folded form used here (exact up to fp reassociation):
    pre_g = h @ (Wh @ Wg) + T[x_t]     where T[v] = (E[v]+b) @ Wg + bg
so the recurrence needs only 4 gate matmuls per step plus a row-gather
from a precomputed per-token gate table.

Sharding: tensor-parallel over the unit dim across 8 cores. Core c owns
output units [c*128,(c+1)*128) of every gate. Per step, one AllGather of
the fp32 hidden state h^T (each rank contributes [128,128]) provides the
full contraction input for the next step's gate matmuls. The gate table
(32000 x 512 fp32 per core = (E+b) @ Wg slice + bg) is computed on-device:
a dense near-roofline matmul.

Layouts are transposed (units on partitions, batch on free dim); the
gathered token rows are transposed on the PE via out = lhsT.T @ I as the
PSUM accumulation base.

The 512 steps are split into two chained NEFF executions (256 steps each)
because one NEFF cannot hold 511 collectives; the gate table and the
hidden state are handed off device-resident (no host round-trip).
"""

import numpy as np

B = 128
S = 512
UNITS = 1024
VOCAB = 32000
NCORES = 8
UC = UNITS // NCORES       # 128 units per core
KCH = UNITS // 128         # 8 contraction chunks
VCH = VOCAB // 128         # 250 vocab chunks
GC = 4 * UC                # 512 packed gate cols per core, order (f,i,o,c)


def _build_chunk(s_steps: int, with_table: bool):
    """One NEFF: optionally compute the gate table, then run s_steps of the
    recurrence. I/O (per core):
      in:  wq [128,KCH*GC] f32, x [128,s] i32, ident [128,128] f32,
           h_in [128,128] f32 (all-zero for the first chunk)
           if with_table: wg [128,KCH*GC] f32, eT [VCH,128,KCH*128] f32,
                          bgrow [1,GC] f32, ones [1,128] f32
           else:          tab_in [VOCAB,GC] f32
      out: y [s,128,128] f32, h_out [128,128] f32,
           tab [VOCAB,GC] f32 (only written when with_table)
    """
    import concourse.bass as bass
    import concourse.mybir as mybir
    import concourse.tile as tile
    from concourse import bacc

    f32 = mybir.dt.float32
    i32 = mybir.dt.int32
    AF = mybir.ActivationFunctionType
    ALU = mybir.AluOpType

    nc = bacc.Bacc(
        "TRN2",
        target_bir_lowering=False,
        debug=False,
        num_devices=NCORES,
        enable_partition_id=False,
    )

    wq = nc.dram_tensor("wq", [128, KCH * GC], f32, kind="ExternalInput")
    xin = nc.dram_tensor("x", [128, s_steps], i32, kind="ExternalInput")
    ident = nc.dram_tensor("ident", [128, 128], f32, kind="ExternalInput")
    if with_table:
        wg = nc.dram_tensor("wg", [128, KCH * GC], f32, kind="ExternalInput")
        eT = nc.dram_tensor("eT", [VCH, 128, KCH * 128], f32, kind="ExternalInput")
        bgrow = nc.dram_tensor("bgrow", [1, GC], f32, kind="ExternalInput")
        ones = nc.dram_tensor("ones", [1, 128], f32, kind="ExternalInput")
        tab = nc.dram_tensor("tab", [VOCAB, GC], f32, kind="ExternalOutput")
    else:
        h_in = nc.dram_tensor("h_in", [128, 128], f32, kind="ExternalInput")
        tab_in = nc.dram_tensor("tab_in", [VOCAB, GC], f32, kind="ExternalInput")
    yout = nc.dram_tensor("y", [s_steps, 128, 128], f32, kind="ExternalOutput")
    h_out = nc.dram_tensor("h_out", [128, 128], f32, kind="ExternalOutput")

    with tile.TileContext(nc) as tc:
        with (
            tc.tile_pool(name="const", bufs=1) as cpool,
            tc.tile_pool(name="agbuf", bufs=2, space="DRAM") as agpool,
            tc.tile_pool(name="psum2", bufs=2, space="PSUM") as pp2,
            tc.tile_pool(name="work", bufs=2) as wpool,
            tc.tile_pool(name="gbuf", bufs=3) as gpool,
        ):
            id_sb = cpool.tile([128, 128], f32, name="id_sb")
            nc.sync.dma_start(id_sb[:], ident[:])
            x_sb = cpool.tile([128, s_steps], i32, name="x_sb")
            nc.sync.dma_start(x_sb[:], xin[:])
            wq_sb = cpool.tile([128, KCH * GC], f32, name="wq_sb")
            nc.sync.dma_start(wq_sb[:], wq[:])

            if with_table:
                # ---------- phase 1: tab = (E+b) @ WgPack + bg ----------
                with (
                    tc.tile_pool(name="wgp", bufs=1) as wgpool,
                    tc.tile_pool(name="etile", bufs=4) as epool,
                    tc.tile_pool(name="tstage", bufs=4) as tpool,
                    tc.tile_pool(name="psum1", bufs=2, space="PSUM") as pp1,
                ):
                    wg_sb = wgpool.tile([128, KCH * GC], f32, name="wg_sb")
                    nc.sync.dma_start(wg_sb[:], wg[:])
                    bg_sb = wgpool.tile([1, GC], f32, name="bg_sb")
                    nc.sync.dma_start(bg_sb[:], bgrow[:])
                    ones_sb = wgpool.tile([1, 128], f32, name="ones_sb")
                    nc.sync.dma_start(ones_sb[:], ones[:])
                    for v in range(VCH):
                        et = epool.tile([128, KCH * 128], f32, name="et")
                        nc.gpsimd.dma_start(et[:], eT[v])
                        ps1 = pp1.tile([128, GC], f32, space="PSUM", name="ps_tab")
                        for l in range(KCH):
                            nc.tensor.matmul(
                                ps1[:],
                                lhsT=et[:, l * 128 : (l + 1) * 128],
                                rhs=wg_sb[:, l * GC : (l + 1) * GC],
                                start=(l == 0),
                                stop=False,
                            )
                        # + broadcast gate bias: ones.T @ bg_row (K=1 matmul)
                        nc.tensor.matmul(
                            ps1[:], lhsT=ones_sb[:, :], rhs=bg_sb[:, :],
                            start=False, stop=True,
                        )
                        ts = tpool.tile([128, GC], f32, name="ts")
                        nc.vector.tensor_copy(ts[:], ps1[:])
                        nc.gpsimd.dma_start(tab[v * 128 : (v + 1) * 128, :], ts[:])
                tab_src = tab
            else:
                tab_src = tab_in

            # ---------- initial h ----------
            if with_table:
                h_prev = None          # h == 0 at t=0
                hT_next = None
            else:
                h_prev = wpool.tile([128, 128], f32, name="h_new")
                nc.sync.dma_start(h_prev[:], h_in[:])
                agin0 = agpool.tile([128, 128], f32, name="agin")
                nc.scalar.dma_start(agin0[:], h_prev[:])
                agout0 = agpool.tile(
                    [NCORES * 128, 128], f32, addr_space="Shared", name="agout"
                )
                nc.gpsimd.collective_compute(
                    "AllGather",
                    mybir.AluOpType.bypass,
                    replica_groups=[list(range(NCORES))],
                    ins=[agin0.opt()],
                    outs=[agout0.opt()],
                )
                hT_next = wpool.tile([128, KCH * 128], f32, name="hT_next")
                for half in range(2):
                    src = agout0[half * 512 : (half + 1) * 512, :].rearrange(
                        "(k p) n -> p k n", p=128
                    )
                    dst = hT_next[:, half * 512 : (half + 1) * 512].rearrange(
                        "p (k n) -> p k n", k=4
                    )
                    nc.scalar.dma_start(dst, src)

            # ---------- recurrence ----------
            for t in range(s_steps):
                g_sb = gpool.tile([128, GC], f32, name="g_sb")
                nc.gpsimd.indirect_dma_start(
                    out=g_sb[:],
                    out_offset=None,
                    in_=tab_src[:],
                    in_offset=bass.IndirectOffsetOnAxis(ap=x_sb[:, t : t + 1], axis=0),
                )

                ps = pp2.tile([128, GC], f32, space="PSUM", name="ps_gate")
                # out = lhsT.T @ I transposes the gathered rows into [u, n]
                # as the PSUM accumulation base
                for g in range(4):
                    nc.tensor.matmul(
                        ps[:, g * 128 : (g + 1) * 128],
                        lhsT=g_sb[:, g * 128 : (g + 1) * 128],
                        rhs=id_sb[:],
                        start=True,
                        stop=(h_prev is None),
                    )
                if h_prev is not None:
                    hT = hT_next
                    for k in range(KCH):
                        for g in range(4):
                            nc.tensor.matmul(
                                ps[:, g * 128 : (g + 1) * 128],
                                lhsT=wq_sb[:, k * GC + g * 128 : k * GC + (g + 1) * 128],
                                rhs=hT[:, k * 128 : (k + 1) * 128],
                                start=False,
                                stop=(k == KCH - 1),
                            )

                # gate order in packed cols: (f, i, o, c)
                fio = wpool.tile([128, 3 * 128], f32, name="fio")
                nc.scalar.activation(fio[:], ps[:, 0 : 3 * 128], AF.Sigmoid)
                cc = wpool.tile([128, 128], f32, name="cc")
                nc.scalar.activation(cc[:], ps[:, 3 * 128 : 4 * 128], AF.Tanh)

                t2 = wpool.tile([128, 128], f32, name="t2")
                nc.vector.tensor_tensor(
                    out=t2[:], in0=fio[:, 128:256], in1=cc[:], op=ALU.mult
                )
                h_new = wpool.tile([128, 128], f32, name="h_new")
                if h_prev is None:
                    nc.vector.tensor_copy(h_new[:], t2[:])
                else:
                    t1 = wpool.tile([128, 128], f32, name="t1")
                    nc.vector.tensor_tensor(
                        out=t1[:], in0=fio[:, 0:128], in1=h_prev[:], op=ALU.mult
                    )
                    nc.vector.tensor_tensor(
                        out=h_new[:], in0=t1[:], in1=t2[:], op=ALU.add
                    )

                if t < s_steps - 1:
                    # broadcast h_new to every core (fp32 AllGather)
                    agin = agpool.tile([128, 128], f32, name="agin")
                    nc.scalar.dma_start(agin[:], h_new[:])
                    agout = agpool.tile(
                        [NCORES * 128, 128], f32, addr_space="Shared", name="agout"
                    )
                    nc.gpsimd.collective_compute(
                        "AllGather",
                        mybir.AluOpType.bypass,
                        replica_groups=[list(range(NCORES))],
                        ins=[agin.opt()],
                        outs=[agout.opt()],
                    )
                    hT_next = wpool.tile([128, KCH * 128], f32, name="hT_next")
                    for half in range(2):
                        src = agout[half * 512 : (half + 1) * 512, :].rearrange(
                            "(k p) n -> p k n", p=128
                        )
                        dst = hT_next[:, half * 512 : (half + 1) * 512].rearrange(
                            "p (k n) -> p k n", k=4
                        )
                        nc.scalar.dma_start(dst, src)
                else:
                    nc.scalar.dma_start(h_out[:], h_new[:])

                th = wpool.tile([128, 128], f32, name="th")
                nc.scalar.activation(th[:], h_new[:], AF.Tanh)
                y_sb = wpool.tile([128, 128], f32, name="y_sb")
                nc.vector.tensor_tensor(
                    out=y_sb[:], in0=fio[:, 256:384], in1=th[:], op=ALU.mult
                )
                nc.gpsimd.dma_start(yout[t], y_sb[:])

                h_prev = h_new

    nc.finalize()
    return nc


def _prep_host(inputs: dict):
    """Host-side shard/pack. Returns (shared arrays, per-core arrays)."""
    f32 = np.float32
    x = np.ascontiguousarray(np.asarray(inputs["x"], dtype=np.int32))
    E = np.asarray(inputs["E"], dtype=f32)
    Wh = np.asarray(inputs["Wh"], dtype=f32)
    b = np.asarray(inputs["b"], dtype=f32)
    Ws = {g: np.asarray(inputs["W" + g], dtype=f32) for g in "fioc"}
    bs = {g: np.asarray(inputs["b" + g], dtype=f32) for g in "fioc"}

    E2 = (E + b[None, :]).astype(f32)  # fold discrete-transform bias into E
    eT_host = np.ascontiguousarray(
        E2.reshape(VCH, 128, KCH, 128).transpose(0, 3, 2, 1).reshape(VCH, 128, KCH * 128)
    )
    ident = np.eye(128, dtype=f32)
    ones = np.ones((1, 128), dtype=f32)
    h0 = np.zeros((128, 128), dtype=f32)

    per_core = []
    for c in range(NCORES):
        sl = slice(c * UC, (c + 1) * UC)
        wgp = np.concatenate([Ws[g][:, sl] for g in "fioc"], axis=1)  # [1024, GC]
        wg_host = np.ascontiguousarray(
            wgp.reshape(KCH, 128, GC).transpose(1, 0, 2).reshape(128, KCH * GC)
        )
        wqp = (Wh @ wgp).astype(f32)  # fp32 fold of Wh into the gate weights
        wq_host = np.ascontiguousarray(
            wqp.reshape(KCH, 128, GC).transpose(1, 0, 2).reshape(128, KCH * GC)
        )
        bgr = np.concatenate([bs[g][sl] for g in "fioc"]).astype(f32)[None, :]
        per_core.append({"wq": wq_host, "wg": wg_host, "bgrow": bgr})
    shared = {"x": x, "eT": eT_host, "ident": ident, "ones": ones, "h_in": h0}
    return shared, per_core


def _make_exec(nc):
    """jit-compiled 8-core shard_map executor for a finalized Bacc module."""
    import jax
    from jax.sharding import Mesh, PartitionSpec
    from jax.experimental.shard_map import shard_map
    import concourse.mybir as mybir
    from concourse import bass2jax

    bass2jax.install_neuronx_cc_hook()

    in_names, out_names, out_avals, out_shapes = [], [], [], []
    for alloc in nc.m.functions[0].allocations:
        if not isinstance(alloc, mybir.MemoryLocationSet):
            continue
        name = alloc.memorylocations[0].name
        if alloc.kind == "ExternalInput":
            in_names.append(name)
        elif alloc.kind == "ExternalOutput":
            out_names.append(name)
            shape = tuple(alloc.tensor_shape)
            dtype = mybir.dt.np(alloc.dtype)
            out_avals.append(jax.core.ShapedArray(shape, dtype))
            out_shapes.append((shape, dtype))
    n_params = len(in_names)
    n_outs = len(out_avals)
    all_names = in_names + out_names

    def _body(*args):
        outs = bass2jax._bass_exec_p.bind(
            *args,
            out_avals=tuple(out_avals),
            in_names=tuple(all_names),
            out_names=tuple(out_names),
            lowering_input_output_aliases=(),
            sim_require_finite=True,
            sim_require_nnan=True,
            nc=nc,
        )
        return tuple(outs)

    devices = jax.devices()[:NCORES]
    mesh = Mesh(np.asarray(devices), ("core",))
    sharded = jax.jit(
        shard_map(
            _body,
            mesh=mesh,
            in_specs=(PartitionSpec("core"),) * (n_params + n_outs),
            out_specs=(PartitionSpec("core"),) * n_outs,
            check_rep=False,
        ),
        donate_argnums=tuple(range(n_params, n_params + n_outs)),
        keep_unused=True,
    )
    return sharded, in_names, out_names, out_shapes, mesh


_CACHE = {}


def _get_execs(chunks):
    key = tuple(chunks)
    if key not in _CACHE:
        execs = []
        for ci, s_chunk in enumerate(chunks):
            nc = _build_chunk(s_chunk, with_table=(ci == 0))
            execs.append(_make_exec(nc))
        _CACHE[key] = execs
    return _CACHE[key]


def _chunk_sizes(s_steps):
    if s_steps <= 256:
        return [s_steps]
    n_chunks = (s_steps + 255) // 256
    base = s_steps // n_chunks
    chunks = [base] * n_chunks
    chunks[-1] += s_steps - sum(chunks)
    return chunks


def _run(inputs: dict, s_steps: int = S, timing=None):
    import time

    import jax
    from jax.sharding import NamedSharding, PartitionSpec

    chunks = _chunk_sizes(s_steps)
    execs = _get_execs(chunks)
    shared, per_core = _prep_host(inputs)

    mesh = execs[0][4]
    sh = NamedSharding(mesh, PartitionSpec("core"))

    def put(arr):
        if isinstance(arr, list):
            cat = np.concatenate([np.asarray(a) for a in arr], axis=0)
        else:
            cat = np.concatenate([np.asarray(arr)] * NCORES, axis=0)
        return jax.device_put(cat, sh)

    staged = {
        "wq": put([m["wq"] for m in per_core]),
        "wg": put([m["wg"] for m in per_core]),
        "bgrow": put([m["bgrow"] for m in per_core]),
        "eT": put(shared["eT"]),
        "ident": put(shared["ident"]),
        "ones": put(shared["ones"]),
        "h_in": put(shared["h_in"]),
    }
    x = shared["x"][:, :s_steps]
    xs = []
    off = 0
    for s_chunk in chunks:
        xs.append(put(np.ascontiguousarray(x[:, off : off + s_chunk])))
        off += s_chunk

    t0 = time.time()
    ys = []
    tab_dev = None
    h_dev = staged["h_in"]
    for ci, s_chunk in enumerate(chunks):
        sharded, in_names, out_names, out_shapes, _ = execs[ci]
        cur = dict(staged)
        cur["x"] = xs[ci]
        cur["h_in"] = h_dev
        if ci > 0:
            cur["tab_in"] = tab_dev
        zeros = [
            jax.device_put(np.zeros((NCORES * sh0[0], *sh0[1:]), dt0), sh)
            for (sh0, dt0) in out_shapes
        ]
        args = [cur[n] for n in in_names] + zeros
        outs = sharded(*args)
        om = dict(zip(out_names, outs))
        ys.append(om["y"])
        h_dev = om["h_out"]
        if "tab" in om:
            tab_dev = om["tab"]
    jax.block_until_ready(ys + [h_dev])
    t1 = time.time()
    if timing is not None:
        timing.append(t1 - t0)

    out = np.empty((B, s_steps, UNITS), dtype=np.float32)
    off = 0
    for ci, s_chunk in enumerate(chunks):
        yc = np.asarray(ys[ci]).reshape(NCORES, s_chunk, 128, 128)
        for c in range(NCORES):
            out[:, off : off + s_chunk, c * UC : (c + 1) * UC] = yc[c].transpose(
                2, 0, 1
            )
        off += s_chunk
    return out


def kernel(**inputs) -> np.ndarray:
    return _run(inputs, S)
